# revision 16
# baseline (speedup 1.0000x reference)
"""Trainium2 Bass kernel for the MANN network (LSTM scan + memory-write scan).

Self-contained: hardcodes all shapes. kernel(**inputs) takes full numpy inputs
and returns the full [128, 40] final memory matrix.

Structure (single-core program, replicated on 8 cores via SPMD):
  Phase A (batch): GXT[p, jj, t] = gate pre-activations from x/y (PE matmuls -> DRAM)
  Loop over 32 chunks of 128 steps:
    B(c): 128 sequential LSTM steps (tanh-only nonlinearities, fp16 W_hh stationary)
    C(c): chunk keys/sigma batch matmuls + per-chunk broadcast tiles
    D(c-1): 128 sequential memory-update steps in slot-major [128,40] layout:
            the M write is a fused scalar_tensor_tensor (per-partition ww), the
            score/rownorm reductions are fused stt+accum reductions along
            the free axis, softmax exp uses the real Exp table (same ACT set as
            tanh/sign) with a per-partition rsqrt scale, and the softmax sum
            comes back as a [128,1] broadcast via a ones-matrix matmul.
            The rsqrt Newton state is refreshed every 2nd step and consumed one
            step stale (validated to 9e-5 rel err).
"""

import sys

import numpy as np

# concourse (Bass) lives in the TRN RL repo; make it importable regardless of cwd
for _p in ("/opt/trn_rl_repo", "/root/.axon_site/_ro/trn_rl_repo"):
    try:
        import concourse  # noqa: F401
        break
    except ImportError:
        if _p not in sys.path:
            sys.path.insert(0, _p)

T, D, F, H, NS, KD = 4096, 512, 256, 200, 128, 40
TC = 128                  # steps per chunk
NCH = T // TC             # 32 chunks
G4P = 1024                # padded gate vector (4 gates x 256)
QUAKE_F = 1597463007.0    # 0x5f3759df as float
N_CORES = 8


# ---------------------------------------------------------------- host prep --
def _prep(inputs):
    f32 = np.float32
    x = np.ascontiguousarray(inputs["x_train"], f32)
    y = np.ascontiguousarray(inputs["y_train"], f32)
    W_in = np.asarray(inputs["W_in"], f32)
    b_in = np.asarray(inputs["b_in"], f32)
    W_ih = np.asarray(inputs["W_ih"], f32)
    W_hh = np.asarray(inputs["W_hh"], f32)
    b_ih = np.asarray(inputs["b_ih"], f32)
    b_hh = np.asarray(inputs["b_hh"], f32)
    W_k = np.asarray(inputs["W_k"], f32)
    b_k = np.asarray(inputs["b_k"], f32)
    W_s = np.asarray(inputs["W_s"], f32)
    b_s = np.asarray(inputs["b_s"], f32)

    # Gate reorder (i, f, gg, o) -> (i, f, o, gg); sigmoid gates scaled by 0.5
    # (sigmoid(v) = 0.5*tanh(0.5 v)+0.5), pad each gate 200 -> 256 rows.
    gate_src = [0, 1, 3, 2]
    scale = [0.5, 0.5, 0.5, 1.0]
    b_tot = b_ih + b_hh
    Wtil = np.zeros((G4P, F + 2), f32)   # cols 0:256 = x feats, 256 = y, 257 = bias
    Whhp = np.zeros((G4P, H), f32)
    for g in range(4):
        src = gate_src[g]
        rows = slice(256 * g, 256 * g + H)
        Wtil[rows, 0:F + 1] = scale[g] * W_ih[200 * src:200 * src + H, :]
        Wtil[rows, F + 1] = scale[g] * b_tot[200 * src:200 * src + H]
        Whhp[rows, :] = scale[g] * W_hh[200 * src:200 * src + H, :]

    watil_t = np.ascontiguousarray(Wtil.T)                     # [258, 1024] f32
    # h is carried as 2h (h2 = (tanh_o+1)*tanh(c)); absorb the 0.5 into the
    # weights that consume h: W_hh and the hid rows of W_k/W_s.
    whhT = np.ascontiguousarray(0.5 * Whhp.T).astype(np.float16)   # [200, 1024] fp16
    # keys/sigma weights: rows 0:200 hid, 200:224 zero pad, 224 bias; col 40 scaled W_s
    wks = np.zeros((225, KD + 1), f32)
    wks[0:H, 0:KD] = 0.5 * W_k
    wks[224, 0:KD] = b_k
    wks[0:H, KD] = 0.25 * W_s[:, 0]
    wks[224, KD] = 0.5 * b_s[0]
    wks = wks.astype(np.float16)

    ysh1 = np.zeros((2, T), f32)          # row0 = y_shift, row1 = ones
    ysh1[0, 1:] = y[:-1, 0]
    ysh1[1, :] = 1.0
    return {
        "x_train": x.astype(np.float16),
        "ysh1": ysh1.astype(np.float16),
        "watil_t": watil_t.astype(np.float16),
        "whht": whhT,
        "wks": wks,
        "w_in": np.ascontiguousarray(W_in).astype(np.float16),
        "b_in": np.ascontiguousarray(b_in.reshape(2, 128)),   # [m, p] -> load as [128,2]
    }


# ------------------------------------------------------------- bass program --
def build(nc, tc):
    import concourse.bass as bass
    from concourse import mybir
    from concourse.bass import ds

    f32 = mybir.dt.float32
    f16 = mybir.dt.float16
    u32 = mybir.dt.uint32
    AF = mybir.ActivationFunctionType
    OP = mybir.AluOpType

    x_d = nc.dram_tensor("x_train", [T, D], f16, kind="ExternalInput")
    y_d = nc.dram_tensor("ysh1", [2, T], f16, kind="ExternalInput")
    watil_d = nc.dram_tensor("watil_t", [F + 2, G4P], f16, kind="ExternalInput")
    whht_d = nc.dram_tensor("whht", [H, G4P], f16, kind="ExternalInput")
    wks_d = nc.dram_tensor("wks", [225, KD + 1], f16, kind="ExternalInput")
    win_d = nc.dram_tensor("w_in", [D, F], f16, kind="ExternalInput")
    bin_d = nc.dram_tensor("b_in", [2, 128], f32, kind="ExternalInput")
    m_out = nc.dram_tensor("m_out", [NS, KD], f32, kind="ExternalOutput")
    gxt_d = nc.dram_tensor("gxt", [128, 8, T + 3 * TC], f16)  # cols TC..TC+T real
    hid_d = nc.dram_tensor("hid_all", [NCH * 225, TC], f16)   # per-chunk hiddens

    from contextlib import ExitStack
    stack = ExitStack()

    singles = stack.enter_context(tc.tile_pool(name="singles", bufs=1))

    # ---------------- persistent loop tiles ----------------
    whh_lo = singles.tile([128, G4P], f16)
    whh_hi = singles.tile([72, G4P], f16)
    wks_lo = singles.tile([128, KD + 1], f16)
    wks_hi = singles.tile([97, KD + 1], f16)
    ident128 = singles.tile([128, 128], f32)
    ones_row32 = singles.tile([1, 128], f32)
    ones128sq = singles.tile([128, 128], f32)
    ident16 = singles.tile([128, 128], f16)

    # paired-chain LSTM state (4 pairs x 2 chains)
    NPAIR = 4
    h16p = [singles.tile([128, 2, 2], f16, tag=f"h16p{p}", name=f"h16p{p}")
            for p in range(NPAIR)]
    tgxp = [singles.tile([128, 2, 10], f32, tag=f"tgxp{p}", name=f"tgxp{p}")
            for p in range(NPAIR)]
    gsump = [singles.tile([128, 2, 4], f32, tag=f"gsp{p}", name=f"gsp{p}")
             for p in range(NPAIR)]
    thcp = [singles.tile([128, 2, 2], f32, tag=f"thcp{p}", name=f"thcp{p}")
            for p in range(NPAIR)]

    # D-scan state, slot-major
    Ms = singles.tile([NS, KD], f32)        # memory [slot, key]
    norm2 = singles.tile([128, 1], f32)
    rn = singles.tile([128, 1], f32)        # rsqrt(row-norm^2), newton state
    rn2 = singles.tile([128, 1], f32)
    nt2 = singles.tile([128, 1], f32)
    nt3 = singles.tile([128, 1], f32)
    qu1 = singles.tile([128, 1], u32)
    qf1 = singles.tile([128, 1], f32)
    qf2 = singles.tile([128, 1], f32)
    qy0 = singles.tile([128, 1], u32)
    e_col = singles.tile([128, 1], f32)
    rs_bc = singles.tile([128, 1], f32)
    t1c = singles.tile([128, 1], f32)
    ww = singles.tile([128, 1], f32)
    scores = singles.tile([128, 1], f32)
    p_col = singles.tile([128, 1], f32)
    scr = singles.tile([128, 2, KD], f32)   # ttr full-output scratch (ping)
    scr2 = singles.tile([128, KD], f32)     # ttr scratch for norm2

    # per-chain gx wave tiles (ping-pong) and hidden-write tiles
    gxk = [[singles.tile([128, 8, TC], f16, tag=f"gx{k}_{p}", name=f"gx{k}_{p}")
            for p in range(2)] for k in range(8)]
    hidw_a = [[singles.tile([128, TC], f16, tag=f"hwa{k}_{p}", name=f"hwa{k}_{p}")
               for p in range(2)] for k in range(8)]
    hidw_b = [[singles.tile([97, TC], f16, tag=f"hwb{k}_{p}", name=f"hwb{k}_{p}")
               for p in range(2)] for k in range(8)]
    # D-phase chunk hidden tiles (ping-pong, loaded back from DRAM)
    hidc_a = [singles.tile([128, TC], f16, tag=f"ha{p}", name=f"ha{p}") for p in range(2)]
    hidc_b = [singles.tile([97, TC], f16, tag=f"hb{p}", name=f"hb{p}") for p in range(2)]
    keysc = [singles.tile([128, KD + 1], f16, tag=f"kc{p}", name=f"kc{p}") for p in range(2)]
    knsc = [singles.tile([128, KD], f16, tag=f"kn{p}", name=f"kn{p}") for p in range(2)]
    sigrow = [singles.tile([1, 128], f32, tag=f"sr{p}", name=f"sr{p}") for p in range(2)]
    sig_bc = [singles.tile([128, TC], f32, tag=f"sb{p}", name=f"sb{p}") for p in range(2)]
    omsig_bc = [singles.tile([128, TC], f32, tag=f"ob{p}", name=f"ob{p}") for p in range(2)]
    alpha_bc = [singles.tile([128, TC], f32, tag=f"ab{p}", name=f"ab{p}") for p in range(2)]
    krdt = [singles.tile([1, TC, KD], f16, tag=f"kr{p}", name=f"kr{p}") for p in range(2)]
    knrdt = [singles.tile([1, TC, KD], f16, tag=f"knr{p}", name=f"knr{p}") for p in range(2)]
    kb_all = [singles.tile([128, TC, KD], f16, tag=f"kba{p}", name=f"kba{p}") for p in range(2)]
    knb_all = [singles.tile([128, TC, KD], f16, tag=f"knba{p}", name=f"knba{p}") for p in range(2)]
    sigpad = singles.tile([128, 32], f32)
    sigtr = singles.tile([128, 32], f32)
    apad = singles.tile([128, 32], f32)
    atr = singles.tile([128, 32], f32)
    arow_sb = [singles.tile([1, 128], f32, tag=f"ar{p}", name=f"ar{p}")
               for p in range(2)]

    # ---------------- static init ----------------
    nc.sync.dma_start(whh_lo[:], whht_d[0:128, :])
    nc.sync.dma_start(whh_hi[:], whht_d[128:200, :])
    nc.sync.dma_start(wks_lo[:], wks_d[0:128, :])
    nc.sync.dma_start(wks_hi[:], wks_d[128:225, :])
    nc.vector.memset(ident128[:], 1.0)
    nc.gpsimd.affine_select(ident128[:], ident128[:], [[-1, 128]], OP.is_equal, 0.0,
                            base=0, channel_multiplier=1)
    nc.vector.tensor_copy(ident16[:], ident128[:])
    nc.vector.memset(ones_row32[:], 1.0)
    nc.vector.memset(ones128sq[:], 1.0)
    for p in range(4):
        nc.vector.memset(h16p[p][:], 0.0)
        nc.vector.memset(tgxp[p][:], 0.0)
    nc.vector.memset(gxk[0][0][:], 0.0)     # chain 0 warm-up: zero gate inputs
    for k in range(8):
        for p in range(2):
            nc.vector.memset(hidw_b[k][p][:], 0.0)
            nc.vector.memset(hidw_b[k][p][96:97, :], 1.0)
    nc.vector.memset(Ms[:], 1e-6)
    nc.vector.memset(rn[:], 1.0)
    nc.vector.memset(e_col[:], 0.0)
    nc.vector.memset(e_col[0:1, :], 1.0)
    nc.vector.memset(sigpad[:], 0.0)
    nc.vector.memset(apad[:], 0.0)

    # ---------------- phase A: GXT ----------------
    with tc.tile_pool(name="pha1", bufs=1) as pha1, \
         tc.tile_pool(name="pha", bufs=3) as pha, \
         tc.tile_pool(name="pha_ps", bufs=2, space="PSUM") as pha_ps:
        xT = [pha1.tile([128, T], f16, tag=f"xT{k}", name=f"xT{k}") for k in range(4)]
        for k in range(4):
            nc.sync.dma_start(xT[k][:], x_d[:, 128 * k:128 * (k + 1)].rearrange("t d -> d t"))
        win_sb = pha1.tile([128, 4, F], f16)
        nc.sync.dma_start(win_sb[:], win_d.rearrange("(k p) f -> p k f", p=128))
        binc = pha1.tile([128, 2], f32)
        nc.sync.dma_start(binc[:], bin_d.rearrange("m p -> p m"))
        wat0 = pha1.tile([128, G4P], f16)
        wat1 = pha1.tile([128, G4P], f16)
        wat2 = pha1.tile([2, G4P], f16)
        nc.sync.dma_start(wat0[:], watil_d[0:128, :])
        nc.sync.dma_start(wat1[:], watil_d[128:256, :])
        nc.sync.dma_start(wat2[:], watil_d[256:258, :])

        xys0 = pha1.tile([128, T], f16)
        xys1 = pha1.tile([128, T], f16)
        xys2 = pha1.tile([2, T], f16)
        nc.sync.dma_start(xys2[:], y_d[:])

        # xsT = W_in.T @ x.T  (+ b_in)
        for m in range(2):
            dst = xys0 if m == 0 else xys1
            for n in range(8):
                ps = pha_ps.tile([128, 512], f32, tag="psA")
                for k in range(4):
                    nc.tensor.matmul(ps[:], win_sb[:, k, 128 * m:128 * (m + 1)],
                                     xT[k][:, 512 * n:512 * (n + 1)],
                                     start=(k == 0), stop=(k == 3))
                nc.vector.tensor_scalar(dst[:, 512 * n:512 * (n + 1)], ps[:],
                                        binc[:, m:m + 1], None, OP.add)

        # GXT = Wtil_aug.T-slices @ xysT -> DRAM (n outer: early cols first)
        for n in range(8):
            for jj in range(8):
                ps = pha_ps.tile([128, 512], f32, tag="psA")
                nc.tensor.matmul(ps[:], wat0[:, 128 * jj:128 * (jj + 1)],
                                 xys0[:, 512 * n:512 * (n + 1)], start=True, stop=False)
                nc.tensor.matmul(ps[:], wat1[:, 128 * jj:128 * (jj + 1)],
                                 xys1[:, 512 * n:512 * (n + 1)], start=False, stop=False)
                nc.tensor.matmul(ps[:], wat2[:, 128 * jj:128 * (jj + 1)],
                                 xys2[:, 512 * n:512 * (n + 1)], start=False, stop=True)
                stg = pha.tile([128, 512], f16, tag="stgA")
                nc.vector.tensor_copy(stg[:], ps[:])
                nc.sync.dma_start(gxt_d[:, jj, TC + 512 * n:TC + 512 * (n + 1)], stg[:])

    # loop-phase PSUM (allocated after phase A pools close)
    psingles = stack.enter_context(tc.tile_pool(name="psingles", bufs=1, space="PSUM"))
    gPp = [psingles.tile([128, 16], f32, tag=f"gPp{p}", name=f"gPp{p}")
           for p in range(4)]
    kraw = psingles.tile([128, KD + 1], f32)
    bcps = psingles.tile([128, 128], f32)
    se_bc = psingles.tile([128, 1], f32)
    nc.vector.memset(se_bc[:], 1.0)

    # ---------------- emitters ----------------
    def emit_Bp_step(p, par, s):
        """One lockstep LSTM step for chain pair p (chains 2p, 2p+1)."""
        gPt = gPp[p]
        tg, gs, th, h2 = tgxp[p], gsump[p], thcp[p], h16p[p]
        for ci in range(2):
            k = 2 * p + ci
            nc.tensor.matmul(gPt[:, 8 * ci:8 * ci + 8], ident16[:],
                             gxk[k][par][:, :, s], start=True, stop=False)
            for kc in range(2):
                slab = whh_lo if kc == 0 else whh_hi
                rhs = h2[:, ci, 0:1] if kc == 0 else h2[0:72, ci, 1:2]
                for jj in range(8):
                    nc.tensor.matmul(gPt[:, 8 * ci + jj:8 * ci + jj + 1],
                                     slab[:, 128 * jj:128 * (jj + 1)], rhs,
                                     start=False, stop=(kc == 1 and jj == 7))
        # both chains' gate tanh in one ACT instruction
        nc.scalar.activation(tg[:, :, 0:8], gPt[:], AF.Tanh)
        for ci in range(2):
            nc.scalar.activation(tg[:, ci, 6:7], tg[:, ci, 6:7], AF.Identity, scale=0.5)
            nc.scalar.activation(tg[:, ci, 7:8], tg[:, ci, 7:8], AF.Identity, scale=0.5)
        nc.vector.scalar_tensor_tensor(gs[:], tg[:, :, 0:4], 1.0, tg[:, :, 6:10],
                                       OP.add, OP.mult)
        for ci in range(2):
            nc.scalar.activation(th[:, ci, 0:1], gs[:, ci, 2:3], AF.Tanh, scale=0.5,
                                 bias=gs[:, ci, 0:1])
            nc.scalar.activation(th[:, ci, 1:2], gs[:, ci, 3:4], AF.Tanh, scale=0.5,
                                 bias=gs[:, ci, 1:2])
        nc.vector.scalar_tensor_tensor(h2[:], tg[:, :, 4:6], 1.0, th[:],
                                       OP.add, OP.mult)
        for ci in range(2):
            k = 2 * p + ci
            nc.gpsimd.tensor_copy(hidw_a[k][par][:, s:s + 1], h2[:, ci, 0:1])
            nc.gpsimd.tensor_copy(hidw_b[k][par][0:72, s:s + 1], h2[0:72, ci, 1:2])
        nc.vector.scalar_tensor_tensor(tg[:, :, 8:10], gs[:, :, 2:4], 0.5,
                                       gs[:, :, 0:2], OP.mult, OP.add)

    def emit_wave(par, s0=0):
        for s in range(s0, TC):
            for p in range(4):
                emit_Bp_step(p, par, s)

    def emit_gx_dma(par, wexpr):
        # chain k, wave w reads gxt cols [512k + 128w, +TC)
        for k in range(8):
            nc.sync.dma_start(gxk[k][par][:],
                              gxt_d[:, :, ds(512 * k + TC * wexpr, TC)])

    def emit_hidw_dma(par, wexpr):
        # store wave outputs: chunk slot = 4k + (w-1)
        for k in range(8):
            nc.sync.dma_start(hid_d[ds((4 * k + wexpr - 1) * 225, 128), :],
                              hidw_a[k][par][:])
            nc.sync.dma_start(hid_d[ds((4 * k + wexpr - 1) * 225 + 128, 97), :],
                              hidw_b[k][par][:])

    def emit_hid_load(cexpr):
        par = cexpr % 2 if isinstance(cexpr, int) else None
        assert par is not None or True
        if isinstance(cexpr, int):
            nc.sync.dma_start(hidc_a[cexpr % 2][:],
                              hid_d[cexpr * 225:cexpr * 225 + 128, :])
            nc.sync.dma_start(hidc_b[cexpr % 2][:],
                              hid_d[cexpr * 225 + 128:cexpr * 225 + 225, :])

    def emit_hid_load_dyn(par, cexpr):
        nc.sync.dma_start(hidc_a[par][:], hid_d[ds(cexpr * 225, 128), :])
        nc.sync.dma_start(hidc_b[par][:], hid_d[ds(cexpr * 225 + 128, 97), :])

    def emit_krdt_dma(par):
        # step-major key / sign(key) rows for the partition broadcasts
        nc.sync.dma_start(krdt[par][:], keysc[par][:, 0:KD])
        nc.sync.dma_start(knrdt[par][:], knsc[par][:])

    def emit_bcast(c):
        # chunk kb/knb broadcast tiles on the otherwise-idle Pool engine
        par = c % 2
        nc.gpsimd.partition_broadcast(kb_all[par][:], krdt[par][0:1, :, :])
        nc.gpsimd.partition_broadcast(knb_all[par][:], knrdt[par][0:1, :, :])

    def emit_Ck(c):
        par = c % 2
        nc.tensor.matmul(kraw[:], hidc_a[par][:], wks_lo[:], start=True, stop=False)
        nc.tensor.matmul(kraw[:], hidc_b[par][:], wks_hi[:], start=False, stop=True)
        nc.scalar.activation(keysc[par][:], kraw[:], AF.Tanh)
        nc.scalar.activation(knsc[par][:], keysc[par][:, 0:KD], AF.Sign)
        # sigma + alpha rows via 32x32 stream-transposes
        nc.vector.tensor_scalar(sigpad[:, 0:1], keysc[par][:, KD:KD + 1], 0.5, 0.5,
                                OP.mult, OP.add)
        nc.vector.tensor_reduce(apad[:, 0:1], keysc[par][:, 0:KD],
                                mybir.AxisListType.X, OP.add,
                                apply_absolute_value=True)
        nc.vector.transpose(sigtr[:], sigpad[:])
        nc.vector.transpose(atr[:], apad[:])
        for i in range(4):
            nc.gpsimd.tensor_copy(sigrow[par][0:1, 32 * i:32 * (i + 1)],
                                  sigtr[32 * i:32 * i + 1, 0:32])
            nc.gpsimd.tensor_copy(arow_sb[par][0:1, 32 * i:32 * (i + 1)],
                                  atr[32 * i:32 * i + 1, 0:32])
        emit_krdt_dma(par)

    def emit_Cbc(c):
        # slot-broadcast tiles [128, TC] for chunk c (PE outer product; Pool copies)
        par = c % 2
        nc.tensor.matmul(bcps[:], ones_row32[:], arow_sb[par][:], start=True, stop=True)
        nc.scalar.activation(alpha_bc[par][:], bcps[:], AF.Copy)
        nc.tensor.matmul(bcps[:], ones_row32[:], sigrow[par][:], start=True, stop=True)
        nc.scalar.activation(sig_bc[par][:], bcps[:], AF.Copy)
        nc.gpsimd.tensor_scalar(omsig_bc[par][:], sig_bc[par][:], -1.0, 1.0,
                                OP.mult, OP.add)

    def emit_D_step(c, s, cold=False):
        par = c % 2
        sp = s % 2
        kb = kb_all[par][:, s, :]
        knb = knb_all[par][:, s, :]
        # p = rowdot(Ms, knb) with Ms as of step s-1 (runs during prev exp)
        nc.vector.scalar_tensor_tensor(scr[:, sp, :], Ms[:], 1.0, knb,
                                       OP.mult, OP.mult, accum_out=p_col[:])
        # softmax chain: rs -> ww -> scores
        nc.vector.reciprocal(rs_bc[:], se_bc[:])
        nc.vector.scalar_tensor_tensor(t1c[:], rs_bc[:], sig_bc[par][:, s:s + 1],
                                       e_col[:], OP.mult, OP.mult)
        nc.vector.tensor_scalar(ww[:], t1c[:], omsig_bc[par][:, s:s + 1], None, OP.add)
        nc.vector.scalar_tensor_tensor(scores[:], ww[:], alpha_bc[par][:, s:s + 1],
                                       p_col[:], OP.mult, OP.add)
        # memory write: Ms += ww * kb   (per-partition scalar ww)
        nc.vector.scalar_tensor_tensor(Ms[:], kb, ww[:], Ms[:], OP.mult, OP.add)
        if cold:
            # exact row-norm^2 + quake rsqrt + 3 newton iters, fresh rn
            nc.vector.scalar_tensor_tensor(scr2[:], Ms[:], 1.0, Ms[:],
                                           OP.mult, OP.mult, accum_out=norm2[:])
            nc.vector.tensor_scalar(qu1[:], norm2.bitcast(u32)[:], 1, None,
                                    OP.logical_shift_right)
            nc.vector.tensor_copy(qf1[:], qu1[:])
            nc.vector.tensor_scalar(qf2[:], qf1[:], -1.0, QUAKE_F, OP.mult, OP.add)
            nc.vector.tensor_copy(qy0[:], qf2[:])
            nc.vector.tensor_copy(rn[:], qy0.bitcast(f32)[:])
            for _ in range(3):
                nc.vector.tensor_mul(rn2[:], rn[:], rn[:])
                nc.vector.tensor_mul(nt2[:], rn2[:], norm2[:])
                nc.vector.tensor_scalar(nt3[:], nt2[:], -0.5, 1.5, OP.mult, OP.add)
                nc.vector.tensor_mul(rn[:], rn[:], nt3[:])
        # softmax exp with per-partition rsqrt scale; sum broadcast via PE
        nc.scalar.activation(e_col[:], scores[:], AF.Exp, scale=rn[:])
        nc.tensor.matmul(se_bc[:], ones128sq[:], e_col[:], start=True, stop=True)
        if not cold and sp == 0:
            # refresh rn (consumed one step stale; emitted after the exp above)
            nc.vector.scalar_tensor_tensor(scr2[:], Ms[:], 1.0, Ms[:],
                                           OP.mult, OP.mult, accum_out=norm2[:])
            nc.vector.tensor_mul(rn2[:], rn[:], rn[:])
            nc.vector.tensor_mul(nt2[:], rn2[:], norm2[:])
            nc.vector.tensor_scalar(nt3[:], nt2[:], -0.5, 1.5, OP.mult, OP.add)
            nc.vector.tensor_mul(rn[:], rn[:], nt3[:])

    def emit_D_chunk(c, cold=False):
        for s in range(TC):
            emit_D_step(c, s, cold and s < 48)

    # ---------------- B phase: 5 waves of 8 interleaved chains ----------------
    # wave w (w=0 warm-up): chain k steps over gxt cols [512k+128w, +128).
    # Chain 0's wave-0 gx tile is the zero memset (exact fixed point at state 0).
    for k in range(1, 8):
        nc.sync.dma_start(gxk[k][0][:], gxt_d[:, :, 512 * k:512 * k + TC])
    # wave 0 (par 0) + prefetch wave 1
    for k in range(8):
        nc.sync.dma_start(gxk[k][1][:], gxt_d[:, :, 512 * k + TC:512 * k + 2 * TC])
    emit_wave(0)
    # waves 1..4 via hardware loop (2 waves per body)
    with tc.For_i(0, 2) as i:
        w1 = 2 * i + 1
        emit_gx_dma(0, w1 + 1)
        emit_wave(1)
        emit_hidw_dma(1, w1)
        w2 = 2 * i + 2
        emit_gx_dma(1, w2 + 1)
        emit_wave(0)
        emit_hidw_dma(0, w2)

    # ---------------- D phase: 32 sequential memory chunks ----------------
    # sub-body cc: [hid(cc+3)] [Ck(cc+2)] [bcast(cc+1)] [D(cc)] [Cbc(cc+1)]
    # keys run 2 chunks ahead, slot-broadcast tiles 1 chunk ahead of the scan.
    emit_hid_load(0)
    emit_hid_load(1)
    emit_Ck(0)
    emit_hid_load(2)
    emit_Ck(1)
    emit_bcast(0)
    emit_Cbc(0)
    # cc = 0, 1 (cold chunk 0 stays out of the hardware loop)
    emit_hid_load(3)
    emit_Ck(2)
    emit_bcast(1)
    emit_D_chunk(0, cold=True)
    emit_Cbc(1)
    emit_hid_load(4)
    emit_Ck(3)
    emit_bcast(2)
    emit_D_chunk(1)
    emit_Cbc(2)
    with tc.For_i(0, 13) as j:
        cc = 2 * j + 2
        emit_hid_load_dyn(1, cc + 3)
        emit_Ck(4)                     # parity 0 == (cc+2) % 2
        emit_bcast(3)                  # parity 1 == (cc+1) % 2
        emit_D_chunk(2)                # parity 0 == cc % 2
        emit_Cbc(3)                    # parity 1 == (cc+1) % 2
        emit_hid_load_dyn(0, cc + 4)
        emit_Ck(5)
        emit_bcast(4)
        emit_D_chunk(3)
        emit_Cbc(4)
    # tail: cc = 28..31
    emit_hid_load(31)
    emit_Ck(30)
    emit_bcast(29)
    emit_D_chunk(28)
    emit_Cbc(29)
    emit_Ck(31)
    emit_bcast(30)
    emit_D_chunk(29)
    emit_Cbc(30)
    emit_bcast(31)
    emit_D_chunk(30)
    emit_Cbc(31)
    emit_D_chunk(31)

    # output: Ms is already [128 slots, 40 keys]
    nc.sync.dma_start(m_out[:], Ms[:])

    stack.close()
    return m_out


_CACHE = {}


def _get_program():
    if "nc" not in _CACHE:
        import concourse.bacc as bacc
        import concourse.tile as tile
        nc = bacc.Bacc("TRN2", target_bir_lowering=False, debug=False)
        with tile.TileContext(nc) as tc:
            build(nc, tc)
        nc.compile()
        _CACHE["nc"] = nc
    return _CACHE["nc"]


def kernel(**inputs) -> np.ndarray:
    from concourse import bass_utils
    nc = _get_program()
    in_map = _prep(inputs)
    res = bass_utils.run_bass_kernel_spmd(
        nc, [dict(in_map) for _ in range(N_CORES)], core_ids=list(range(N_CORES))
    )
    return res.results[0]["m_out"]

